# revision 14
# baseline (speedup 1.0000x reference)
"""Trainium2 Bass kernel for nn_ArcEmbedding (embedding lookup + 3-axis RoPE).

Reference computation (per token t in batch b):
    e = emb_table[id]                       # [768]
    theta = [xn*invf, yn*invf, tn*invf]     # [384], xn = x/max(max_b(x),1) etc
    out[0:384]   = e[0:384]*cos(theta) - e[384:768]*sin(theta)
    out[384:768] = e[384:768]*cos(theta) + e[0:384]*sin(theta)

Kernel strategy (data-parallel over batch, 4 batches per NeuronCore, 8 cores):
  Polar refactor: with e1=e[0:384], e2=e[384:768],
      r_s  = sign(e1)*sqrt(e1^2+e2^2)
      phi0 = atan(e2/e1) in (-pi/2, pi/2)
      psi  = phi0 + theta                  (one 384-col matmul per tile)
      out[0:384]   = r_s * cos(psi) = r_s * sin(-psi + pi/2)
      out[384:768] = r_s * sin(psi)
  The host packs onehot(id) + raw x + raw y + normalized t as ONE
  [67, S] DRAM tensor per batch (single input DMA); the per-batch
  1/max normalization is folded into the x/y angle rows of a tiny
  per-batch copy of the stationary-side table.

  QUAD pipeline: tokens are processed 512 at a time (4 tiles of 128,
  interleaved host-side so one output DMA writes 6144 contiguous bytes
  per partition). Per quad:
    - PE: 4 psi matmuls into one [P,2048] PSUM tile (384 cols per 2KB
      bank), then 4 amplitude-gather matmuls into a second [P,2048]
      PSUM tile (the "psi" tag's other buffer).
    - ACT: ONE Sin instruction over all 4 tiles (1536 elems) + ONE
      cos-via-sin(-x+pi/2) instruction. Batching 4 tiles per
      instruction halves the ~420ns per-instruction ACT overhead.
    - DVE: ONE tensor_copy downcasting the amplitudes PSUM f32 ->
      SBUF bf16, then ONE multiply over tiles 0..2 with all-bf16
      packed operands (2x_1p DVE perf mode).
    - GPSIMD: multiplies tile 3 (it cannot touch PSUM, but the bf16
      amplitude copy + trig live in SBUF).
    - SP: one 6KB/partition output DMA.
"""

import numpy as np

B, S, H, V = 32, 4096, 768, 64
P = 128
NCORES = 8
BPC = B // NCORES            # batches per core
NT = S // P                  # 128-token tiles per batch
NQUAD = NT // 4              # 4-tile quads per batch
HALF = H // 2                # 384
DA = HALF // 3               # 128 freqs per axis
KL = V + 3                   # lhsT rows: onehot + x + y + t
ROPE_BASE = 10000.0

_INVF = (1.0 / (ROPE_BASE ** (np.arange(DA, dtype=np.float64) / DA))).astype(
    np.float32
)
_TNORM = (np.arange(S, dtype=np.float64) / (S - 1)).astype(np.float32)
# quad interleave: block of 512 tokens -> [t%4==0 | t%4==1 | t%4==2 | t%4==3]
_PERM = np.arange(S).reshape(NQUAD, P, 4).transpose(0, 2, 1).reshape(S)

_COMPILED = {}
LAST_RESULTS = None


def _build_program():
    import concourse.bacc as bacc
    import concourse.mybir as mybir
    import concourse.tile as tile

    f32 = mybir.dt.float32
    bf16 = mybir.dt.bfloat16
    AF = mybir.ActivationFunctionType
    ALU = mybir.AluOpType

    nc = bacc.Bacc("TRN2", target_bir_lowering=False, debug=False)

    pk_d = nc.dram_tensor("pk", [BPC, KL, S], bf16, kind="ExternalInput")
    xymax_d = nc.dram_tensor("xymax", [BPC, P, 2 * NT], bf16, kind="ExternalInput")
    emb_d = nc.dram_tensor("emb", [V, H], f32, kind="ExternalInput")
    rtail_d = nc.dram_tensor("rhs_tail", [3, HALF], bf16, kind="ExternalInput")
    ident_d = nc.dram_tensor("ident", [P, P], f32, kind="ExternalInput")
    out_d = nc.dram_tensor("out", [BPC, S, H], bf16, kind="ExternalOutput")

    with tile.TileContext(nc) as tc:
        with (
            tc.tile_pool(name="const", bufs=1) as cpool,
            tc.tile_pool(name="batch", bufs=BPC) as bpool,
            tc.tile_pool(name="work", bufs=4) as wpool,
            tc.tile_pool(name="psum", bufs=2, space="PSUM") as ppool,
        ):
            # ---------------- input DMAs (Sync program order) -------------
            emb_sb = cpool.tile([V, H], f32)
            nc.sync.dma_start(out=emb_sb[:], in_=emb_d[:])
            ident_t = cpool.tile([P, P], f32)
            nc.sync.dma_start(out=ident_t[:], in_=ident_d[:])
            mxins, Ls = [], []
            for b in range(BPC):
                mxin = bpool.tile([P, 2 * NT], bf16, tag="mxin", name=f"mxin{b}")
                nc.sync.dma_start(out=mxin[:], in_=xymax_d[b])
                mxins.append(mxin)
            for b in range(BPC):
                L = bpool.tile([KL, S], bf16, tag="bigL", name=f"L{b}")
                Ls.append(L)
            # only batch 0 loads up front: a scheduler stage barrier right
            # after the prologue waits for ALL in-flight DMAs, so the other
            # batches' big loads are issued from inside the quad loop.
            # split 64+3: a 64-row DMA spreads across the DMA engines,
            # a 67-row one degrades to a single-engine chain
            # batch 0's load is split into column chunks so the first quads
            # can start as soon as their tokens land
            NCH = 4
            for ch in range(NCH):
                c0, c1 = ch * (S // NCH), (ch + 1) * (S // NCH)
                nc.sync.dma_start(out=Ls[0][0:V, c0:c1], in_=pk_d[0, 0:V, c0:c1])
                nc.sync.dma_start(
                    out=Ls[0][V:KL, c0:c1], in_=pk_d[0, V:KL, c0:c1]
                )

            # ---------------- one-time table prep ----------------
            # shared table: [psi cols 0:384 | r cols 384:768]
            # rows 0:64 = phi / r_s gather rows, rows 64:67 = angle rows
            # (unscaled; per-batch scaled copies of the psi half below)
            rhs_t = cpool.tile([KL, H], bf16)
            nc.vector.memset(rhs_t[:], 0.0)
            nc.sync.dma_start(out=rhs_t[64:KL, 0:HALF], in_=rtail_d[:])
            halfpi = cpool.tile([P, 1], f32)
            nc.vector.memset(halfpi[:], float(np.pi / 2))

            # phi branch first: it gates the per-batch psi tables
            e1 = emb_sb[:, 0:HALF]
            e2 = emb_sb[:, HALF:H]
            einv = cpool.tile([V, HALF], f32)
            nc.vector.reciprocal_approx_fast(out=einv[:], in_=e1)
            quo = cpool.tile([V, HALF], f32)
            nc.vector.tensor_tensor(out=quo[:], in0=e2, in1=einv[:], op=ALU.mult)
            phi = cpool.tile([V, HALF], f32)
            nc.scalar.activation(out=phi[:], in_=quo[:], func=AF.Arctan)
            nc.scalar.copy(out=rhs_t[0:V, 0:HALF], in_=phi[:])
            # tiny dummy Sin pulls the trig ACT table load off the critical
            # path (it would otherwise happen right before the first quad)
            dummy = cpool.tile([P, 1], f32)
            nc.scalar.activation(out=dummy[:], in_=halfpi[:], func=AF.Sin)

            # r_s = sign(e1)*sqrt(e1^2+e2^2) = e1*sqrt(1+q^2), reusing q from
            # the phi branch; the +1 rides ACT Sqrt's bias port
            q2 = cpool.tile([V, HALF], f32)
            nc.vector.tensor_tensor(out=q2[:], in0=quo[:], in1=quo[:], op=ALU.mult)
            rmag = cpool.tile([V, HALF], f32)
            nc.scalar.activation(out=rmag[:], in_=q2[:], func=AF.Sqrt, bias=1.0)
            rsg = cpool.tile([V, HALF], f32)
            nc.vector.tensor_tensor(out=rsg[:], in0=rmag[:], in1=e1, op=ALU.mult)
            nc.vector.tensor_copy(out=rhs_t[0:V, HALF:H], in_=rsg[:])

            # ---------------- per-batch normalization ----------------
            # max over batch -> 1/max folded into the x/y angle rows of a
            # small per-batch copy of the psi table half
            rhsbs = []
            for b in range(BPC):
                mxin = mxins[b]
                mx2 = bpool.tile([P, 2], f32, tag="mx2", name=f"mx2{b}")
                nc.vector.tensor_reduce(
                    out=mx2[:, 0:1], in_=mxin[:, 0:NT],
                    axis=mybir.AxisListType.X, op=ALU.max,
                )
                nc.vector.tensor_reduce(
                    out=mx2[:, 1:2], in_=mxin[:, NT:2 * NT],
                    axis=mybir.AxisListType.X, op=ALU.max,
                )
                # shares the rg psum slot (prologue-only use)
                pmx = ppool.tile([2, P], f32, tag="rg", name=f"pmx{b}", bufs=1)
                nc.tensor.transpose(out=pmx[:], in_=mx2[:], identity=ident_t[:])
                stg = bpool.tile([2, 4], f32, tag="stg", name=f"stg{b}")
                nc.vector.tensor_reduce(
                    out=stg[:, 0:1], in_=pmx[:],
                    axis=mybir.AxisListType.X, op=ALU.max,
                )
                nc.vector.tensor_scalar(
                    out=stg[:, 1:2], in0=stg[:, 0:1], scalar1=1.0,
                    scalar2=None, op0=ALU.max,
                )
                nc.vector.reciprocal(out=stg[:, 2:3], in_=stg[:, 1:2])
                rhsb = bpool.tile([KL, HALF], bf16, tag="rhsb", name=f"rhsb{b}")
                if b == 0:
                    # batch 0 gates the first matmul: build straight from phi
                    # on DVE, skipping the phi -> rhs_t -> rhsb round-trip
                    nc.vector.tensor_copy(out=rhsb[0:V, :], in_=phi[:])
                    nc.vector.tensor_copy(
                        out=rhsb[V:KL, :], in_=rhs_t[V:KL, 0:HALF]
                    )
                else:
                    nc.vector.tensor_copy(out=rhsb[:], in_=rhs_t[:, 0:HALF])
                nc.vector.tensor_scalar(
                    out=rhsb[64:66, :], in0=rhs_t[64:66, 0:HALF],
                    scalar1=stg[:, 2:3], scalar2=None, op0=ALU.mult,
                )
                rhsbs.append(rhsb)

            # bf16 amplitude plane for one batch: 32 tiles x 384 cols
            rgball = cpool.tile([P, NT * HALF], bf16)

            # ---------------- main loop: quads of 4 tiles ----------------
            # The multiply + output DMA for quad q are emitted one quad
            # later, so they never sit on the PE->ACT critical path.
            pending = None        # (b, q, tgq4, pool-slice args)

            def flush(pend):
                b, q, tgq4 = pend
                w0 = q * 4 * P
                rgb3 = rgball[:, q * 4 * HALF:(q + 1) * 4 * HALF].rearrange(
                    "p (t h) -> p t h", t=4, h=HALF
                )
                od4 = out_d[b, w0:w0 + 4 * P, :].rearrange(
                    "(p k) h -> p k h", k=4
                )
                # DVE multiplies tiles 0..2, GPSIMD tile 3, into separate
                # tiles with separate DMAs so the slow GPSIMD mult never
                # gates the main output DMA.
                ota = wpool.tile([P, 3 * H], bf16, tag="ota", bufs=6)
                ota4 = ota[:].rearrange("p (t two h) -> p t two h", t=3, two=2)
                rgbv = rgb3[:, 0:3, None, :].to_broadcast([P, 3, 2, HALF])
                nc.vector.tensor_tensor(
                    out=ota4[:], in0=rgbv, in1=tgq4[:, 0:3, :, :],
                    op=ALU.mult,
                )
                nc.sync.dma_start(out=od4[:, 0:3, :], in_=ota[:])
                otb = wpool.tile([P, H], bf16, tag="otb", bufs=6)
                otb3 = otb[:].rearrange("p (two h) -> p two h", two=2)
                rgbv3 = rgb3[:, 3, None, :].to_broadcast([P, 2, HALF])
                nc.gpsimd.tensor_tensor(
                    out=otb3, in0=rgbv3, in1=tgq4[:, 3, :, :],
                    op=ALU.mult,
                )
                nc.sync.dma_start(out=od4[:, 3, :], in_=otb[:])

            def rg_pair(p):
                # amplitude gather + downcast for global pair p (2 tiles),
                # through a single [P,1024] psum buffer (2 banks). Pairs are
                # emitted one quad late, sandwiched between psi matmul
                # groups, so the cast latency never stalls the PE queue.
                bb = p // (2 * NQUAD)
                tt0 = (p % (2 * NQUAD)) * 2
                Lp = Ls[bb]
                rgp = ppool.tile([P, 1024], f32, tag="rg", name="rgp", bufs=1)
                for t in range(2):
                    nc.tensor.matmul(
                        rgp[:, t * 512:t * 512 + HALF],
                        Lp[:, (tt0 + t) * P:(tt0 + t + 1) * P],
                        rhs_t[:, HALF:H], start=True, stop=True,
                    )
                rgv = rgp[:].rearrange("p (t s) -> p t s", t=2, s=512)[
                    :, :, 0:HALF
                ]
                rgb = rgball[:, tt0 * HALF:(tt0 + 2) * HALF]
                nc.vector.tensor_copy(
                    out=rgb.rearrange("p (t h) -> p t h", t=2, h=HALF),
                    in_=rgv,
                )

            Q = 0
            for b in range(BPC):
                L = Ls[b]
                rhsb = rhsbs[b]
                for q in range(NQUAD):
                    if q == 0 and b + 1 < BPC:
                        nc.sync.dma_start(
                            out=Ls[b + 1][0:V, :], in_=pk_d[b + 1, 0:V]
                        )
                        nc.sync.dma_start(
                            out=Ls[b + 1][V:KL, :], in_=pk_d[b + 1, V:KL]
                        )
                    w0 = q * 4 * P            # quad start token (packed)

                    # psi quad, PACKED: 4 tiles x 384 cols = [P,1536]
                    # (3 PSUM banks) so two quads double-buffer in 6 banks.
                    # A matmul output cannot cross a 512-col PSUM bank
                    # boundary, so tiles 1 and 2 are written in two pieces.
                    # The quad is emitted in two halves with a lagged rg
                    # pair in between.
                    psiq = ppool.tile([P, 1536], f32, tag="psi")

                    def psi_half(ts):
                        for t in ts:
                            lt = L[:, w0 + t * P:w0 + (t + 1) * P]
                            c0 = t * HALF
                            cuts = [0, HALF]
                            if (c0 % 512) + HALF > 512:
                                cuts = [0, 512 - (c0 % 512), HALF]
                            for a, z in zip(cuts[:-1], cuts[1:]):
                                nc.tensor.matmul(
                                    psiq[:, c0 + a:c0 + z],
                                    lt,
                                    rhsb[:, a:z], start=True, stop=True,
                                )

                    psi_half((0, 1))
                    if Q > 0:
                        rg_pair(2 * Q - 2)
                    psi_half((2, 3))

                    # one trig tile per quad: [cos0|sin0|cos1|sin1|...]
                    # (cos = sin(-x + pi/2)); ONE Sin + ONE cos instruction
                    # covering all 4 tiles (contiguous [P,1536] PSUM read)
                    tgq = wpool.tile([P, 4 * H], bf16, tag="tg", bufs=6)
                    tgq4 = tgq[:].rearrange(
                        "p (t two h) -> p t two h", t=4, two=2
                    )
                    psiv = psiq[:].rearrange("p (t h) -> p t h", t=4, h=HALF)
                    nc.scalar.activation(
                        out=tgq4[:, :, 1, :], in_=psiv, func=AF.Sin,
                    )
                    nc.scalar.activation(
                        out=tgq4[:, :, 0, :], in_=psiv, func=AF.Sin,
                        scale=-1.0, bias=halfpi[:],
                    )

                    if Q > 0:
                        rg_pair(2 * Q - 1)
                        flush(pending)
                    pending = (b, q, tgq4)
                    Q += 1
            rg_pair(2 * Q - 2)
            rg_pair(2 * Q - 1)
            flush(pending)

    nc.compile()
    return nc


def _host_inputs(input_ids, coords, emb_table):
    import ml_dtypes

    bf16 = ml_dtypes.bfloat16
    ids = np.asarray(input_ids).astype(np.float32)[:, _PERM]     # [B, S]
    xy = np.asarray(coords).astype(np.float32)[:, _PERM, :]      # [B, S, 2]
    emb = np.asarray(emb_table).astype(np.float32)               # [V, H]
    tnorm = _TNORM[_PERM]

    ident = np.eye(P, dtype=np.float32)
    rtail = np.zeros((3, HALF), dtype=np.float32)
    rtail[0, 0:DA] = _INVF                                   # x angle row
    rtail[1, DA:2 * DA] = _INVF                              # y angle row
    rtail[2, 2 * DA:HALF] = _INVF                            # t angle row
    rtail = rtail.astype(bf16)

    in_maps = []
    for c in range(NCORES):
        bs = slice(c * BPC, (c + 1) * BPC)
        pk = np.empty((BPC, KL, S), dtype=np.float32)
        pk[:, 0:V, :] = (
            ids[bs][:, None, :] == np.arange(V, dtype=np.float32)[None, :, None]
        )
        pk[:, V + 0, :] = xy[bs, :, 0]
        pk[:, V + 1, :] = xy[bs, :, 1]
        pk[:, V + 2, :] = tnorm[None, :]
        xymax = np.empty((BPC, P, 2 * NT), dtype=np.float32)
        xymax[:, :, 0:NT] = xy[bs, :, 0].reshape(BPC, NT, P).transpose(0, 2, 1)
        xymax[:, :, NT:2 * NT] = (
            xy[bs, :, 1].reshape(BPC, NT, P).transpose(0, 2, 1)
        )
        in_maps.append(
            {
                "pk": pk.astype(bf16),
                "xymax": xymax.astype(bf16),
                "emb": emb,
                "rhs_tail": rtail,
                "ident": ident,
            }
        )
    return in_maps


def kernel(input_ids, coords, emb_table):
    global LAST_RESULTS
    from concourse.bass_utils import run_bass_kernel_spmd

    if "nc" not in _COMPILED:
        _COMPILED["nc"] = _build_program()
    nc = _COMPILED["nc"]

    in_maps = _host_inputs(input_ids, coords, emb_table)
    res = run_bass_kernel_spmd(nc, in_maps, core_ids=list(range(NCORES)))
    LAST_RESULTS = res
    out = np.concatenate(
        [r["out"].astype(np.float32) for r in res.results], axis=0
    )
    return out
